# revision 5
# baseline (speedup 1.0000x reference)
"""Trainium2 Bass kernel for a 2-layer dense-graph GAT encoder (N=4096, H=4).

Math: attention scores are additive: e[i,j,h] = lrelu(e_src[i,h] + e_tgt[j,h]).
exp(lrelu(s)) with s = es + et factors as
    exp(0.2*es) * [ c * max(1, u*v) ],   u = exp(0.8*es_i), v = exp(0.8*et_j),
    c = exp(0.2*et_j),
and the exp(0.2*es_i) factor cancels in the softmax.  So each (j,i) attention
tile is ONE DVE tensor_scalar op:  T[j,i] = max(c_j, (c_j*v_j)*u_i)  applied to
a broadcast tile of u — no N^2 transcendentals.  The N^2 work left is one DVE
op + one PE matmul per 128x512 tile.

Sharding: rows (queries) are split 512/core across 8 cores.  Layer-1
projections (x @ W1) are computed replicated from a pre-transposed x; the
layer-1 output shard h^T (256x512) is AllGathered between layers; layer-2
projections are recomputed replicated from the gathered h^T.  Final output is
returned per-core as (512, 256) row shards and concatenated on host.
"""

import numpy as np
import ml_dtypes

N = 4096
NCORES = 8
NS = N // NCORES          # 512 rows per core
H = 4
D1 = 64                   # layer-1 head dim
HID = 256                 # hidden = H*D1, layer-2 head dim
K1 = 128                  # state_dim
NT = N // 128             # 32 j-tiles
LN_EPS = 1e-5

_BF = ml_dtypes.bfloat16

_compiled = None


def _build():
    import concourse.bass as bass
    import concourse.mybir as mybir
    import concourse.tile as tile
    from concourse import bacc

    f32 = mybir.dt.float32
    bf16 = mybir.dt.bfloat16
    AF = mybir.ActivationFunctionType
    OP = mybir.AluOpType

    nc = bacc.Bacc("TRN2", target_bir_lowering=False, debug=False,
                   num_devices=NCORES)

    # ---- I/O ----
    xT_d = nc.dram_tensor("xT", [K1, N], bf16, kind="ExternalInput")
    xTm_d = nc.dram_tensor("xTm", [K1, NS], bf16, kind="ExternalInput")
    w1_d = nc.dram_tensor("w1a", [K1, HID + 2 * H], bf16, kind="ExternalInput")
    w2_d = nc.dram_tensor("w2a", [HID, H * HID + 2 * H], bf16,
                          kind="ExternalInput")
    gb_d = nc.dram_tensor("gb", [2, HID], f32, kind="ExternalInput")
    out_d = nc.dram_tensor("outT", [NS, HID], f32, kind="ExternalOutput")

    W1C = HID + 2 * H        # 264
    W2C = H * HID + 2 * H    # 1032

    with tile.TileContext(nc) as tc:
        with (
            tc.tile_pool(name="persist", bufs=1) as pp,
            tc.tile_pool(name="xpool", bufs=1) as xp,
            tc.tile_pool(name="work", bufs=1) as wp,
            tc.tile_pool(name="tp", bufs=8) as tp,
            tc.tile_pool(name="dram", bufs=1, space="DRAM") as dram,
        ):
            # ---------- loads ----------
            xT = xp.tile([K1, N], bf16)
            xTm = xp.tile([K1, NS], bf16)
            w1 = pp.tile([K1, W1C], bf16)
            w2 = pp.tile([K1, 2, W2C], bf16)   # k-tiles of W2_aug rows
            nc.sync.dma_start(w1[:], w1_d[:])
            nc.sync.dma_start(w2[:, 0, :], w2_d[0:128, :])
            nc.sync.dma_start(w2[:, 1, :], w2_d[128:256, :])
            nc.sync.dma_start(xTm[:], xTm_d[:])
            for q in range(4):  # chunked for DMA parallelism
                nc.sync.dma_start(xT[:, q * 1024:(q + 1) * 1024],
                                  xT_d[:, q * 1024:(q + 1) * 1024])

            # gamma/beta broadcast rows
            g_row = pp.tile([1, HID], f32)
            b_row = pp.tile([1, HID], f32)
            nc.sync.dma_start(g_row[:], gb_d[0:1, :])
            nc.sync.dma_start(b_row[:], gb_d[1:2, :])
            g_brc = pp.tile([128, HID], f32)
            b_brc = pp.tile([128, HID], f32)
            nc.gpsimd.partition_broadcast(g_brc[:], g_row[:])
            nc.gpsimd.partition_broadcast(b_brc[:], b_row[:])

            # ---------- persistent layer-1 state ----------
            wx1 = pp.tile([128, NT, H, D1 + 1], bf16)     # [.., 0:64]=Wx, 64=ones
            nc.vector.memset(wx1[:, :, :, D1], 1.0)
            esb1 = wp.tile([128, NT, 8], f32)
            c1 = pp.tile([128, NT, H], f32)
            cv1 = pp.tile([128, NT, H], f32)

            with tc.tile_pool(name="psA", bufs=2, space="PSUM") as psA:
                for jt in range(NT):
                    pA = psA.tile([128, W1C], f32, tag="pA")
                    nc.tensor.matmul(pA[:], xT[:, jt * 128:(jt + 1) * 128],
                                     w1[:], start=True, stop=True)
                    nc.vector.tensor_copy(wx1[:, jt, :, 0:D1], pA[:, 0:HID])
                    nc.vector.tensor_copy(esb1[:, jt, :], pA[:, HID:W1C])

                # u1 rows for my shard: e_src1^T via M=1 matmuls
                u1row = []
                for h in range(H):
                    pu = psA.tile([1, NS], f32, tag="pu", bufs=2)
                    nc.tensor.matmul(pu[:], w1[:, HID + h:HID + h + 1],
                                     xTm[:], start=True, stop=True)
                    ur = pp.tile([1, NS], bf16, name=f"u1row{h}",
                                 tag=f"u1row{h}")
                    nc.scalar.activation(ur[:], pu[:], AF.Exp, scale=0.8)
                    u1row.append(ur)

            uv1 = wp.tile([128, NT, 8], f32)
            nc.scalar.activation(uv1[:], esb1[:], AF.Exp, scale=0.8)
            nc.scalar.activation(c1[:], esb1[:, :, H:2 * H], AF.Exp, scale=0.2)
            nc.vector.tensor_tensor(cv1[:], uv1[:, :, H:2 * H], c1[:], OP.mult)

            bu1 = pp.tile([128, H, NS], bf16)
            for h in range(H):
                nc.gpsimd.partition_broadcast(bu1[:, h, :], u1row[h][:])

            # ---------- phase B: layer-1 attention for my 512 rows ----------
            bounce = dram.tile([HID, NS], bf16)
            with tc.tile_pool(name="psB", bufs=1, space="PSUM") as psB:
                pB = [psB.tile([D1 + 1, NS], f32, name=f"pB{h}", tag=f"pB{h}")
                      for h in range(H)]
                for jt in range(NT):
                    for h in range(H):
                        t1 = tp.tile([128, NS], bf16, tag="T1")
                        nc.vector.tensor_scalar(
                            t1[:], bu1[:, h, :], cv1[:, jt, h:h + 1],
                            c1[:, jt, h:h + 1], OP.mult, OP.max)
                        nc.tensor.matmul(pB[h][:], wx1[:, jt, h, :], t1[:],
                                         start=(jt == 0), stop=(jt == NT - 1))
                # epilogue: h1 = elu(num/den), DMA per head into bounce
                for h in range(H):
                    dsc = wp.tile([D1 + 1, NS], f32, tag="dsc", bufs=2)
                    nc.vector.tensor_copy(dsc[D1:D1 + 1, :],
                                          pB[h][D1:D1 + 1, :])
                    den = wp.tile([1, NS], f32, tag="den", bufs=2)
                    nc.sync.dma_start(den[:], dsc[D1:D1 + 1, :])
                    denr = wp.tile([1, NS], f32, tag="denr", bufs=2)
                    nc.vector.reciprocal_approx_fast(denr[:], den[:])
                    brc = wp.tile([D1, NS], f32, tag="brc", bufs=2)
                    nc.gpsimd.partition_broadcast(brc[:], denr[:])
                    o = wp.tile([D1, NS], f32, tag="o", bufs=2)
                    nc.vector.tensor_tensor(o[:], pB[h][0:D1, :], brc[:],
                                            OP.mult)
                    # elu(x) = (relu(x) - 1) + exp(min(x, 0))
                    tmn = wp.tile([D1, NS], f32, tag="tmn", bufs=2)
                    nc.vector.tensor_scalar(tmn[:], o[:], 0.0, None, OP.min)
                    tex = wp.tile([D1, NS], f32, tag="tex", bufs=2)
                    nc.scalar.activation(tex[:], tmn[:], AF.Exp)
                    trl = wp.tile([D1, NS], f32, tag="trl", bufs=2)
                    nc.vector.tensor_scalar(trl[:], o[:], 0.0, -1.0, OP.max,
                                            OP.add)
                    eluh = wp.tile([D1, NS], bf16, tag="eluh", bufs=2)
                    nc.vector.tensor_tensor(eluh[:], tex[:], trl[:], OP.add)
                    nc.sync.dma_start(bounce[h * D1:(h + 1) * D1, :], eluh[:])

            # ---------- phase C: AllGather h^T ----------
            gat = dram.tile([NCORES, HID, NS], bf16, addr_space="Shared")
            nc.gpsimd.collective_compute(
                "AllGather", OP.bypass,
                replica_groups=[list(range(NCORES))],
                ins=[bounce.opt()], outs=[gat.opt()])

            hallT = pp.tile([128, 2, NCORES, NS], bf16)
            for kt in range(2):
                for c in range(NCORES):
                    nc.sync.dma_start(
                        hallT[:, kt, c, :],
                        gat[c, kt * 128:(kt + 1) * 128, :])

            # my own h^T back from local bounce (for u2 rows)
            hmT = wp.tile([128, 2, NS], bf16)
            nc.sync.dma_start(hmT[:, 0, :], bounce[0:128, :])
            nc.sync.dma_start(hmT[:, 1, :], bounce[128:256, :])

            # ---------- persistent layer-2 state ----------
            wx2 = pp.tile([128, NT, H, HID + 1], bf16)
            nc.vector.memset(wx2[:, :, :, HID], 1.0)
            esb2 = wp.tile([128, NT, 8], f32)
            c2 = pp.tile([128, NT, H], f32)
            cv2 = pp.tile([128, NT, H], f32)

            with tc.tile_pool(name="psD", bufs=1, space="PSUM") as psD:
                # u2 rows from local h^T
                u2row = []
                for h in range(H):
                    pu2 = psD.tile([1, NS], f32, tag="pu2", bufs=2)
                    for kt in range(2):
                        nc.tensor.matmul(
                            pu2[:],
                            w2[:, kt, H * HID + h:H * HID + h + 1],
                            hmT[:, kt, :], start=(kt == 0), stop=(kt == 1))
                    ur2 = pp.tile([1, NS], bf16, name=f"u2row{h}",
                                  tag=f"u2row{h}")
                    nc.scalar.activation(ur2[:], pu2[:], AF.Exp, scale=0.8)
                    u2row.append(ur2)

                # Wx2_aug replicated: all 4096 rows
                for jt in range(NT):
                    c8, io = divmod(jt, NT // NCORES)
                    pD1 = psD.tile([128, 512], f32, tag="pD1", bufs=2)
                    pD2 = psD.tile([128, 512], f32, tag="pD2", bufs=2)
                    pD3 = psD.tile([128, 8], f32, tag="pD3", bufs=2)
                    for kt in range(2):
                        lhs = hallT[:, kt, c8, io * 128:(io + 1) * 128]
                        st, sp = (kt == 0), (kt == 1)
                        nc.tensor.matmul(pD1[:], lhs, w2[:, kt, 0:512],
                                         start=st, stop=sp)
                        nc.tensor.matmul(pD2[:], lhs, w2[:, kt, 512:1024],
                                         start=st, stop=sp)
                        nc.tensor.matmul(pD3[:], lhs, w2[:, kt, 1024:1032],
                                         start=st, stop=sp)
                    # interleave into (H, 257) blocks; alternate DVE/ACT
                    if jt % 2 == 0:
                        nc.vector.tensor_copy(wx2[:, jt, 0:2, 0:HID], pD1[:])
                        nc.scalar.copy(wx2[:, jt, 2:4, 0:HID], pD2[:])
                    else:
                        nc.scalar.copy(wx2[:, jt, 0:2, 0:HID], pD1[:])
                        nc.vector.tensor_copy(wx2[:, jt, 2:4, 0:HID], pD2[:])
                    nc.vector.tensor_copy(esb2[:, jt, :], pD3[:])

            uv2 = wp.tile([128, NT, 8], f32)
            nc.scalar.activation(uv2[:], esb2[:], AF.Exp, scale=0.8)
            nc.scalar.activation(c2[:], esb2[:, :, H:2 * H], AF.Exp, scale=0.2)
            nc.vector.tensor_tensor(cv2[:], uv2[:, :, H:2 * H], c2[:], OP.mult)

            bu2 = pp.tile([128, H, NS], bf16)
            for h in range(H):
                nc.gpsimd.partition_broadcast(bu2[:, h, :], u2row[h][:])

            # ---------- phase E: layer-2 attention + LN, i in 2 halves ----------
            EPS_SCALED = LN_EPS * (H * H)   # LN of (H * mean) with eps*H^2
            with tc.tile_pool(name="psE", bufs=1, space="PSUM") as psE:
                for ih in range(2):
                    pE = [[psE.tile([128, HID + 1], f32,
                                    name=f"pE{h}_{m}", tag=f"pE{h}_{m}")
                           for m in range(2)] for h in range(H)]
                    for jt in range(NT):
                        for h in range(H):
                            t2 = tp.tile([128, 256], bf16, tag="T2")
                            nc.vector.tensor_scalar(
                                t2[:], bu2[:, h, ih * 256:(ih + 1) * 256],
                                cv2[:, jt, h:h + 1], c2[:, jt, h:h + 1],
                                OP.mult, OP.max)
                            for m in range(2):
                                nc.tensor.matmul(
                                    pE[h][m][:], t2[:, m * 128:(m + 1) * 128],
                                    wx2[:, jt, h, :],
                                    start=(jt == 0), stop=(jt == NT - 1))
                    for m in range(2):
                        # pack 4 head denominators, one approx-reciprocal
                        dpk = wp.tile([128, H], f32, tag="dpk", bufs=2)
                        for h in range(H):
                            nc.vector.tensor_copy(dpk[:, h:h + 1],
                                                  pE[h][m][:, HID:HID + 1])
                        rr = wp.tile([128, H], f32, tag="rr", bufs=2)
                        nc.vector.reciprocal_approx_fast(rr[:], dpk[:])
                        # sum_h num_h * r_h  (skip 1/H — LN absorbs it)
                        o2 = wp.tile([128, HID], f32, tag="o2", bufs=2)
                        nc.vector.tensor_scalar(o2[:], pE[0][m][:, 0:HID],
                                                rr[:, 0:1], None, OP.mult)
                        for h in range(1, H):
                            th = wp.tile([128, HID], f32, tag="th", bufs=2)
                            nc.vector.tensor_scalar(th[:], pE[h][m][:, 0:HID],
                                                    rr[:, h:h + 1], None,
                                                    OP.mult)
                            nc.vector.tensor_tensor(o2[:], o2[:], th[:],
                                                    OP.add)
                        # layernorm over features (free dim)
                        sm = wp.tile([128, 1], f32, tag="sm", bufs=2)
                        nc.vector.tensor_reduce(sm[:], o2[:],
                                                mybir.AxisListType.X, OP.add)
                        sq = wp.tile([128, HID], f32, tag="sq", bufs=2)
                        ssq = wp.tile([128, 1], f32, tag="ssq", bufs=2)
                        nc.scalar.activation(sq[:], o2[:], AF.Square,
                                             accum_out=ssq[:])
                        mu = wp.tile([128, 1], f32, tag="mu", bufs=2)
                        nc.vector.tensor_scalar(mu[:], sm[:], 1.0 / HID, None,
                                                OP.mult)
                        m2 = wp.tile([128, 1], f32, tag="m2", bufs=2)
                        nc.vector.tensor_scalar(m2[:], ssq[:], 1.0 / HID,
                                                EPS_SCALED, OP.mult, OP.add)
                        mu2 = wp.tile([128, 1], f32, tag="mu2", bufs=2)
                        nc.vector.tensor_tensor(mu2[:], mu[:], mu[:], OP.mult)
                        var = wp.tile([128, 1], f32, tag="var", bufs=2)
                        nc.vector.tensor_tensor(var[:], m2[:], mu2[:],
                                                OP.subtract)
                        lnv = wp.tile([128, 1], f32, tag="lnv", bufs=2)
                        nc.scalar.activation(lnv[:], var[:], AF.Ln)
                        rstd = wp.tile([128, 1], f32, tag="rstd", bufs=2)
                        nc.scalar.activation(rstd[:], lnv[:], AF.Exp,
                                             scale=-0.5)
                        xn = wp.tile([128, HID], f32, tag="xn", bufs=2)
                        nc.vector.tensor_scalar(xn[:], o2[:], mu[:], rstd[:],
                                                OP.subtract, OP.mult)
                        y = wp.tile([128, HID], f32, tag="y", bufs=2)
                        nc.vector.tensor_tensor(y[:], xn[:], g_brc[:], OP.mult)
                        outt = wp.tile([128, HID], f32, tag="outt", bufs=2)
                        nc.vector.tensor_tensor(outt[:], y[:], b_brc[:],
                                                OP.add)
                        r0 = (ih * 2 + m) * 128
                        nc.sync.dma_start(out_d[r0:r0 + 128, :], outt[:])

    nc.compile()
    return nc


def _prep_inputs(x, W1, attn1, W2, attn2, gamma, beta):
    f32 = np.float32
    x = np.asarray(x, f32)
    W1 = np.asarray(W1, f32)
    attn1 = np.asarray(attn1, f32)
    W2 = np.asarray(W2, f32)
    attn2 = np.asarray(attn2, f32)
    gamma = np.asarray(gamma, f32)
    beta = np.asarray(beta, f32)

    vsrc1 = np.stack([W1[:, h * D1:(h + 1) * D1] @ attn1[h, :D1]
                      for h in range(H)], 1)
    vtgt1 = np.stack([W1[:, h * D1:(h + 1) * D1] @ attn1[h, D1:]
                      for h in range(H)], 1)
    w1a = np.concatenate([W1, vsrc1, vtgt1], 1).astype(_BF)

    vsrc2 = np.stack([W2[:, h * HID:(h + 1) * HID] @ attn2[h, :HID]
                      for h in range(H)], 1)
    vtgt2 = np.stack([W2[:, h * HID:(h + 1) * HID] @ attn2[h, HID:]
                      for h in range(H)], 1)
    w2a = np.concatenate([W2, vsrc2, vtgt2], 1).astype(_BF)

    xT = np.ascontiguousarray(x.T).astype(_BF)
    gb = np.stack([gamma, beta], 0).astype(f32)

    in_maps = []
    for c in range(NCORES):
        xTm = np.ascontiguousarray(x[c * NS:(c + 1) * NS, :].T).astype(_BF)
        in_maps.append(dict(xT=xT, xTm=xTm, w1a=w1a, w2a=w2a, gb=gb))
    return in_maps


def _ensure_ntff_hook():
    """Inject the antenv.axon_hooks shim (missing in this image) so
    run_bass_kernel_spmd(trace=True) can capture NTFF profiles via the
    axon .so's C ABI (same mechanism as trn_agent_boot)."""
    import sys
    import types
    import ctypes
    import contextlib

    if "antenv.axon_hooks" in sys.modules:
        return
    so_path = "/opt/axon/libaxon_pjrt.so"
    try:
        lib = ctypes.CDLL(so_path)
    except OSError:
        return
    if not hasattr(lib, "axon_start_nrt_profile"):
        return
    lib.axon_start_nrt_profile.argtypes = [ctypes.POINTER(ctypes.c_int64),
                                           ctypes.c_size_t]
    lib.axon_start_nrt_profile.restype = ctypes.c_int64
    lib.axon_stop_nrt_profile.argtypes = [ctypes.c_char_p]
    lib.axon_stop_nrt_profile.restype = ctypes.c_int64

    @contextlib.contextmanager
    def _hook(output_dir, device_ids):
        import jax
        jax.devices()
        if device_ids:
            ids = (ctypes.c_int64 * len(device_ids))(*device_ids)
            rc = lib.axon_start_nrt_profile(ids, len(device_ids))
        else:
            rc = lib.axon_start_nrt_profile(None, 0)
        if rc != 0:
            raise RuntimeError(f"axon_start_nrt_profile rc={rc}")
        try:
            yield
        finally:
            n = lib.axon_stop_nrt_profile(str(output_dir).encode())
            print(f"ntff profile: {n} file(s) written to {output_dir}")

    mod = types.ModuleType("antenv.axon_hooks")
    mod.get_axon_ntff_profile_hook = lambda: _hook
    mod.set_axon_ntff_profile_hook = lambda h: None
    sys.modules["antenv.axon_hooks"] = mod


def _run(in_maps, trace=False):
    global _compiled
    from concourse.bass_utils import run_bass_kernel_spmd
    if trace:
        _ensure_ntff_hook()
    if _compiled is None:
        _compiled = _build()
    res = run_bass_kernel_spmd(_compiled, in_maps,
                               core_ids=list(range(NCORES)), trace=trace)
    out = np.concatenate([res.results[c]["outT"] for c in range(NCORES)], 0)
    return out.astype(np.float32), res


def kernel(x, W1, attn1, W2, attn2, gamma, beta):
    in_maps = _prep_inputs(x, W1, attn1, W2, attn2, gamma, beta)
    out, _ = _run(in_maps, trace=False)
    return out


def kernel_traced(x, W1, attn1, W2, attn2, gamma, beta):
    """Like kernel() but returns (out, BassKernelResults) with profiling."""
    in_maps = _prep_inputs(x, W1, attn1, W2, attn2, gamma, beta)
    return _run(in_maps, trace=True)
